# revision 2
# baseline (speedup 1.0000x reference)
"""HOPE block kernel for 8 Trainium2 NeuronCores.

Sharding: 8 shards = (batch b in 0..3, sequence half in 0..1), 2048 tokens each.
The linear-attention memory scan is causal per (batch, head); cores owning the
second half of a sequence receive the first half as a "prefix" input and
rebuild the mid-sequence memory state M (= sum_t k_t v_t^T per head) before
scanning their own chunks. Even cores receive a zero prefix, keeping the SPMD
program uniform.

All matmuls run in float32r (fp32 storage, ~12-bit-mantissa PE path, bf16-class
throughput); LayerNorm/residual arithmetic stays fp32.
"""
import sys
if '/opt/trn_rl_repo' not in sys.path:
    sys.path.insert(0, '/opt/trn_rl_repo')

from contextlib import ExitStack
import numpy as np


def _ensure_ntff_hook():
    """Register the axon NTFF profile hook when the image's antenv stub
    lacks `axon_hooks` — otherwise BASS_TRACE runs skip tracing and
    exec_time_ns comes back None. No-op when the real module exists."""
    import types, contextlib, ctypes, os
    try:
        from antenv.axon_hooks import get_axon_ntff_profile_hook  # noqa: F401
        return
    except ImportError:
        pass
    hook = None
    so_path = '/opt/axon/libaxon_pjrt.so'
    if os.path.exists(so_path):
        try:
            lib = ctypes.CDLL(so_path)
        except OSError:
            lib = None
        if lib is not None and hasattr(lib, 'axon_start_nrt_profile'):
            lib.axon_start_nrt_profile.argtypes = [
                ctypes.POINTER(ctypes.c_int64), ctypes.c_size_t]
            lib.axon_start_nrt_profile.restype = ctypes.c_int64
            lib.axon_stop_nrt_profile.argtypes = [ctypes.c_char_p]
            lib.axon_stop_nrt_profile.restype = ctypes.c_int64

            @contextlib.contextmanager
            def hook(output_dir, device_ids):
                import jax
                jax.devices()  # force PJRT init so the .so's client exists
                if device_ids:
                    ids = (ctypes.c_int64 * len(device_ids))(*device_ids)
                    rc = lib.axon_start_nrt_profile(ids, len(device_ids))
                else:
                    rc = lib.axon_start_nrt_profile(None, 0)
                if rc != 0:
                    raise RuntimeError(f"axon_start_nrt_profile rc={rc}")
                try:
                    yield
                finally:
                    n = lib.axon_stop_nrt_profile(str(output_dir).encode())
                    print(f"profile: {n} file(s) written to {output_dir}",
                          file=sys.stderr)

    try:
        import antenv
    except ImportError:
        return
    mod = types.ModuleType('antenv.axon_hooks')
    _h = hook
    mod.get_axon_ntff_profile_hook = lambda: _h
    mod.set_axon_ntff_profile_hook = lambda h: None
    sys.modules['antenv.axon_hooks'] = mod
    antenv.axon_hooks = mod


_ensure_ntff_hook()

import concourse.bass as bass
import concourse.tile as tile
from concourse import mybir
from concourse.bass_utils import run_bass_kernel_spmd
from concourse.masks import make_identity
from concourse.vector_clock import ScopedClock

f32 = mybir.dt.float32
f32r = mybir.dt.float32r
AF = mybir.ActivationFunctionType
ALU = mybir.AluOpType

DIM = 1024
HEADS = 16
HD = 64
B, S = 4, 4096
LEVELS = 3
HID = 4 * DIM
CHUNK = 128
EPS = 1e-5
P = 128

N_CORES = 8
T_OWN = S // 2      # tokens per core
T_PRE = S // 2      # prefix tokens (zeros on even cores)
BLK = 512           # phase-B token block
PRE_BLK = 512      # phase-A token block
D_T = DIM // P      # 8 feature tiles
H_T = HID // P      # 32 hidden tiles
TT_UP = 512         # FFN up-proj token tile
TT_DN = 256         # FFN down-proj token tile

MAX_WAITS = 1


def _split_multi_waits(nc, max_waits=MAX_WAITS):
    """Walrus in this toolchain encodes at most `max_waits` sem waits per
    instruction; split extra waits onto same-engine NOPs placed just before."""
    for f in nc.m.functions:
        for bb in f.blocks:
            insts = list(bb.instructions)
            if not any(
                i.sync_info and i.sync_info.on_wait and len(i.sync_info.on_wait) > max_waits
                for i in insts
            ):
                continue
            new = []
            for inst in insts:
                si = inst.sync_info
                waits = list(si.on_wait) if si and si.on_wait else []
                if len(waits) > max_waits:
                    head, rest = waits[:-max_waits], waits[-max_waits:]
                    while head:
                        chunk, head = head[:max_waits], head[max_waits:]
                        nop = mybir.InstNoOp(name=nc.get_next_instruction_name(), ins=[], outs=[])
                        nop.engine = inst.engine
                        nop.sync_info = mybir.SyncInfo(on_wait=chunk, on_update=[])
                        nc.register_instruction(nop, overwrite=True)
                        new.append(nop)
                    inst.sync_info = mybir.SyncInfo(
                        on_wait=rest, on_update=list(si.on_update) if si.on_update else [])
                new.append(inst)
            bb.instructions = new


def _layernorm_tile(nc, pools, x_t, g_bc, b_bc, eps_t, out_r):
    """LayerNorm of one [128, DIM] fp32 tile -> f32r tile (token-major)."""
    w = pools
    BNF = nc.vector.BN_STATS_FMAX
    nsub = DIM // BNF
    stats = w.tile([P, nsub, nc.vector.BN_STATS_DIM], f32, tag="ln_stats")
    xg = x_t[:].rearrange("p (s f) -> p s f", f=BNF)
    for s_ in range(nsub):
        nc.vector.bn_stats(out=stats[:, s_, :], in_=xg[:, s_, :])
    mv = w.tile([P, nc.vector.BN_AGGR_DIM], f32, tag="ln_mv")
    nc.vector.bn_aggr(out=mv, in_=stats)
    rstd = w.tile([P, 1], f32, tag="ln_rstd")
    nc.scalar.activation(out=rstd, in_=mv[:, 1:2], func=AF.Sqrt, bias=eps_t, scale=1.0)
    nc.vector.reciprocal(out=rstd, in_=rstd)
    tmp = w.tile([P, DIM], f32, tag="ln_tmp")
    nc.vector.tensor_scalar(out=tmp, in0=x_t, scalar1=mv[:, 0:1], scalar2=rstd,
                            op0=ALU.subtract, op1=ALU.mult)
    nc.vector.tensor_mul(out=tmp, in0=tmp, in1=g_bc)
    nc.vector.tensor_add(out=out_r, in0=tmp, in1=b_bc)


def _transpose_block(nc, sb_pool, ps_pool, ident, src_r, n_tok_tiles, out_tile, out_map):
    """PE-transpose token-major f32r [n_tok_tiles*128, DIM] (given per-tile via
    src_r(t) -> AP [128, DIM]) into feature-major layout via out_map(t, f) -> AP
    [128,128] destination slices."""
    for t in range(n_tok_tiles):
        src = src_r(t)
        for fidx in range(D_T):
            ps = ps_pool.tile([P, P], f32r, tag="tp_ps")
            nc.tensor.transpose(ps, src[:, fidx * P:(fidx + 1) * P], ident)
            nc.scalar.copy(out=out_map(t, fidx), in_=ps)


def build_kernel(t_own=T_OWN, t_pre=T_PRE, debug_outputs=False):
    nc = bass.Bass()

    x_own = nc.dram_tensor("x_own", [t_own, DIM], f32, kind="ExternalInput")
    x_pre = nc.dram_tensor("x_pre", [t_pre, DIM], f32, kind="ExternalInput")
    wq = nc.dram_tensor("wq", [DIM, DIM], f32r, kind="ExternalInput")
    wk = nc.dram_tensor("wk", [DIM, DIM], f32r, kind="ExternalInput")
    wv = nc.dram_tensor("wv", [DIM, DIM], f32r, kind="ExternalInput")
    wo = nc.dram_tensor("wo", [DIM, DIM], f32r, kind="ExternalInput")
    ln1_g = nc.dram_tensor("ln1_g", [DIM], f32, kind="ExternalInput")
    ln1_b = nc.dram_tensor("ln1_b", [DIM], f32, kind="ExternalInput")
    ln2_g = nc.dram_tensor("ln2_g", [DIM], f32, kind="ExternalInput")
    ln2_b = nc.dram_tensor("ln2_b", [DIM], f32, kind="ExternalInput")
    cms_w1 = nc.dram_tensor("cms_w1", [LEVELS, DIM, HID], f32r, kind="ExternalInput")
    cms_b1 = nc.dram_tensor("cms_b1", [LEVELS, HID], f32, kind="ExternalInput")
    cms_w2 = nc.dram_tensor("cms_w2", [LEVELS, HID, DIM], f32r, kind="ExternalInput")
    cms_b2 = nc.dram_tensor("cms_b2", [LEVELS, DIM], f32, kind="ExternalInput")
    maskT = nc.dram_tensor("maskT", [CHUNK, CHUNK], f32, kind="ExternalInput")
    out = nc.dram_tensor("out", [t_own, DIM], f32, kind="ExternalOutput")
    x2_dbg = None
    if debug_outputs:
        x2_dbg = nc.dram_tensor("x2_dbg", [t_own, DIM], f32, kind="ExternalOutput")

    n_own_t = t_own // P           # 128-token tiles
    n_blk = t_own // BLK           # phase-B blocks
    n_pre_blk = t_pre // PRE_BLK   # phase-A blocks
    n_tt_up = t_own // TT_UP
    n_tt_dn = t_own // TT_DN

    with tile.TileContext(nc) as tc, ExitStack() as top:
        dram = top.enter_context(tc.tile_pool(name="dram", bufs=1, space="DRAM"))
        x2_d = dram.tile([n_own_t, P, DIM], f32)
        hT_a = dram.tile([P, D_T, t_own], f32r)
        hT_b = dram.tile([P, D_T, t_own], f32r)
        upg_d = dram.tile([P, H_T, t_own], f32r)

        consts = top.enter_context(tc.tile_pool(name="consts", bufs=1))
        ident = consts.tile([P, P], f32r)
        ident_f = consts.tile([P, P], f32)
        make_identity(nc, ident_f)
        nc.vector.tensor_copy(out=ident, in_=ident_f)

        # pools live only through phases A+B (attention); freed before the FFN
        ab_stack = ExitStack()
        ab_consts = ab_stack.enter_context(tc.tile_pool(name="ab_consts", bufs=1))
        eps_t = ab_consts.tile([P, 1], f32)
        nc.vector.memset(eps_t, EPS)
        mask_t = ab_consts.tile([CHUNK, CHUNK], f32)
        nc.sync.dma_start(out=mask_t, in_=maskT.ap())
        g1 = ab_consts.tile([P, DIM], f32)
        b1 = ab_consts.tile([P, DIM], f32)
        g2 = ab_consts.tile([P, DIM], f32)
        b2 = ab_consts.tile([P, DIM], f32)
        nc.sync.dma_start(out=g1, in_=ln1_g.ap()[None, :].partition_broadcast(P).opt())
        nc.sync.dma_start(out=b1, in_=ln1_b.ap()[None, :].partition_broadcast(P).opt())
        nc.sync.dma_start(out=g2, in_=ln2_g.ap()[None, :].partition_broadcast(P).opt())
        nc.sync.dma_start(out=b2, in_=ln2_b.ap()[None, :].partition_broadcast(P).opt())

        # persistent attention memory state; head h lives at partitions
        # (h%2)*HD .. +HD so that reads share the base partition of qcT/kcT
        mt_pool = ab_stack.enter_context(tc.tile_pool(name="mt", bufs=1))
        Mt_f = [mt_pool.tile([P, HD], f32, name=f"Mt_f{h}", tag=f"Mt_f{h}") for h in range(HEADS)]
        Mt_s = [mt_pool.tile([P, HD], f32r, name=f"Mt_s{h}", tag=f"Mt_s{h}") for h in range(HEADS)]
        for h in range(HEADS):
            nc.vector.memset(Mt_f[h], 0.0)

        ln_w = ab_stack.enter_context(tc.tile_pool(name="ln_w", bufs=1))

        # ---------------- Phase A: prefix -> Mt ----------------
        for blk in range(n_pre_blk):
            with ExitStack() as ph:
                sb = ph.enter_context(tc.tile_pool(name="A_sb", bufs=1))
                ps = ph.enter_context(tc.tile_pool(name="A_ps", bufs=2, space="PSUM"))
                wstream = ph.enter_context(tc.tile_pool(name="A_w", bufs=2))
                ntt = PRE_BLK // P
                hpT = sb.tile([P, D_T, PRE_BLK], f32r)
                for t in range(ntt):
                    x_t = sb.tile([P, DIM], f32, tag=f"A_x{t % 2}")
                    nc.sync.dma_start(out=x_t, in_=x_pre.ap()[(blk * ntt + t) * P:(blk * ntt + t + 1) * P, :])
                    h_r = sb.tile([P, DIM], f32r, tag=f"A_h{t % 2}")
                    _layernorm_tile(nc, ln_w, x_t, g1, b1, eps_t, h_r)
                    for fidx in range(D_T):
                        tps = ps.tile([P, P], f32r, tag="tp_ps")
                        nc.tensor.transpose(tps, h_r[:, fidx * P:(fidx + 1) * P], ident)
                        nc.scalar.copy(out=hpT[:, fidx, t * P:(t + 1) * P], in_=tps)
                # kc/vc token-major: out[t, f] via lhsT=hpT tiles, rhs=w slices
                kc = sb.tile([P, ntt, DIM], f32r)
                vc = sb.tile([P, ntt, DIM], f32r)
                for (w_in, dst) in ((wk, kc), (wv, vc)):
                    w_all = w_in.ap().rearrange("(kt p) d -> p kt d", p=P)
                    for nh in range(2):
                        w_t = wstream.tile([P, D_T, 512], f32r, tag="A_wt")
                        nc.sync.dma_start(out=w_t, in_=w_all[:, :, nh * 512:(nh + 1) * 512])
                        for m in range(ntt):
                            pst = ps.tile([P, 512], f32, tag="A_pst")
                            for k in range(D_T):
                                nc.tensor.matmul(pst, hpT[:, k, m * P:(m + 1) * P], w_t[:, k, :],
                                                 start=(k == 0), stop=(k == D_T - 1))
                            nc.scalar.copy(out=dst[:, m, nh * 512:(nh + 1) * 512], in_=pst)
                # accumulate Mt += kc^T vc per head, chunks of 128 tokens
                for h in range(HEADS):
                    pb = (h % 2) * HD
                    mt_ps = ps.tile([HD, HD], f32, tag="A_mt")
                    for ch in range(ntt):
                        nc.tensor.matmul(mt_ps, kc[:, ch, h * HD:(h + 1) * HD],
                                         vc[:, ch, h * HD:(h + 1) * HD],
                                         start=(ch == 0), stop=(ch == ntt - 1))
                    nc.vector.tensor_add(out=Mt_f[h][pb:pb + HD, :], in0=Mt_f[h][pb:pb + HD, :],
                                         in1=mt_ps)
        for h in range(HEADS):
            pb = (h % 2) * HD
            nc.scalar.copy(out=Mt_s[h][pb:pb + HD, :], in_=Mt_f[h][pb:pb + HD, :])

        # ---------------- Phase B: own tokens, attention ----------------
        wo_pool = ab_stack.enter_context(tc.tile_pool(name="wo_pool", bufs=1))
        wo_all = wo.ap().rearrange("(kt p) d -> p kt d", p=P)
        wo_ts = []
        for nh in range(2):
            w_t = wo_pool.tile([P, D_T, 512], f32r, tag=f"wo{nh}", name=f"wo_full_{nh}")
            nc.scalar.dma_start(out=w_t, in_=wo_all[:, :, nh * 512:(nh + 1) * 512])
            wo_ts.append(w_t)
        for blk in range(n_blk):
            ntt = BLK // P  # 4 token tiles per block
            tok0 = blk * BLK
            with ExitStack() as ph:
                sb = ph.enter_context(tc.tile_pool(name="B_sb", bufs=1))
                wstream = ph.enter_context(tc.tile_pool(name="B_w", bufs=2))
                with ExitStack() as sub:
                    ps = sub.enter_context(tc.tile_pool(name="B_ps", bufs=2, space="PSUM"))
                    hT_pool = sub.enter_context(tc.tile_pool(name="B_hT", bufs=1))
                    hT = hT_pool.tile([P, D_T, BLK], f32r)
                    for t in range(ntt):
                        x_t = sb.tile([P, DIM], f32, tag=f"B_x{t % 2}")
                        nc.sync.dma_start(out=x_t, in_=x_own.ap()[tok0 + t * P:tok0 + (t + 1) * P, :])
                        h_r = sb.tile([P, DIM], f32r, tag=f"B_h{t % 2}")
                        _layernorm_tile(nc, ln_w, x_t, g1, b1, eps_t, h_r)
                        for fidx in range(D_T):
                            tps = ps.tile([P, P], f32r, tag="tp_ps")
                            nc.tensor.transpose(tps, h_r[:, fidx * P:(fidx + 1) * P], ident)
                            nc.scalar.copy(out=hT[:, fidx, t * P:(t + 1) * P], in_=tps)
                    # qT, kT feature-major [128, D_T, BLK]; v token-major [128, ntt, DIM]
                    qT = sb.tile([P, D_T, BLK], f32r)
                    kT = sb.tile([P, D_T, BLK], f32r)
                    for (w_in, dst) in ((wq, qT), (wk, kT)):
                        w_all = w_in.ap().rearrange("(kt p) d -> p kt d", p=P)
                        for m in range(D_T):
                            w_t = wstream.tile([P, D_T, P], f32r, tag="B_wt")
                            nc.sync.dma_start(out=w_t, in_=w_all[:, :, m * P:(m + 1) * P])
                            pst = ps.tile([P, BLK], f32, tag="B_pst")
                            for k in range(D_T):
                                nc.tensor.matmul(pst, w_t[:, k, :], hT[:, k, :],
                                                 start=(k == 0), stop=(k == D_T - 1))
                            nc.scalar.copy(out=dst[:, m, :], in_=pst)
                    v = sb.tile([P, ntt, DIM], f32r)
                    wv_all = wv.ap().rearrange("(kt p) d -> p kt d", p=P)
                    for nh in range(2):
                        w_t = wstream.tile([P, D_T, 512], f32r, tag="B_wtv")
                        nc.sync.dma_start(out=w_t, in_=wv_all[:, :, nh * 512:(nh + 1) * 512])
                        for m in range(ntt):
                            pst = ps.tile([P, 512], f32, tag="B_pstv")
                            for k in range(D_T):
                                nc.tensor.matmul(pst, hT[:, k, m * P:(m + 1) * P], w_t[:, k, :],
                                                 start=(k == 0), stop=(k == D_T - 1))
                            nc.scalar.copy(out=v[:, m, nh * 512:(nh + 1) * 512], in_=pst)
                # scan
                y = sb.tile([P, ntt, DIM], f32r)
                with ExitStack() as sub:
                    ps_sc = sub.enter_context(tc.tile_pool(name="B_ps_sc", bufs=2, space="PSUM"))
                    ps_y = sub.enter_context(tc.tile_pool(name="B_ps_y", bufs=2, space="PSUM"))
                    ps_mt = sub.enter_context(tc.tile_pool(name="B_ps_mt", bufs=2, space="PSUM"))
                    ps_kc = sub.enter_context(tc.tile_pool(name="B_ps_kc", bufs=2, space="PSUM"))
                    scw = sub.enter_context(tc.tile_pool(name="B_scw", bufs=3))
                    for h in range(HEADS):
                        pb = (h % 2) * HD
                        fi = h // 2
                        for ch in range(ntt):
                            qcT = qT[pb:pb + HD, fi, ch * P:(ch + 1) * P]
                            kcT = kT[pb:pb + HD, fi, ch * P:(ch + 1) * P]
                            vc = v[:, ch, h * HD:(h + 1) * HD]
                            # kc = kcT^T (token-major)
                            kc_ps = ps_kc.tile([P, HD], f32r, tag="kc")
                            nc.tensor.transpose(kc_ps, kcT, ident[pb:pb + HD, pb:pb + HD])
                            kc_s = scw.tile([P, HD], f32r, tag="kc_s")
                            nc.scalar.copy(out=kc_s, in_=kc_ps)
                            # scoresT = kcT^T-contract: out[e,c] = sum_d kcT[d,e] qcT[d,c]
                            sc_ps = ps_sc.tile([P, P], f32, tag="sc")
                            nc.tensor.matmul(sc_ps, kcT, qcT, start=True, stop=True)
                            sc_r = scw.tile([P, P], f32r, tag="sc_r")
                            nc.vector.tensor_mul(out=sc_r, in0=sc_ps, in1=mask_t)
                            # y = scores^T-contract intra + inter
                            y_ps = ps_y.tile([P, HD], f32, tag="y")
                            nc.tensor.matmul(y_ps, sc_r, vc, start=True, stop=False)
                            nc.tensor.matmul(y_ps, qcT, Mt_s[h][pb:pb + HD, :], start=False, stop=True)
                            nc.scalar.copy(out=y[:, ch, h * HD:(h + 1) * HD], in_=y_ps)
                            # Mt += kc^T vc
                            mt_ps = ps_mt.tile([HD, HD], f32, tag="mt")
                            nc.tensor.matmul(mt_ps, kc_s, vc, start=True, stop=True)
                            nc.vector.tensor_add(out=Mt_f[h][pb:pb + HD, :],
                                                 in0=Mt_f[h][pb:pb + HD, :], in1=mt_ps)
                            nc.scalar.copy(out=Mt_s[h][pb:pb + HD, :], in_=Mt_f[h][pb:pb + HD, :])
                # yT, attn-out + residual, LN2, h2T
                with ExitStack() as sub:
                    ps = sub.enter_context(tc.tile_pool(name="B_ps2", bufs=2, space="PSUM"))
                    ps_at = sub.enter_context(tc.tile_pool(name="B_ps_at", bufs=2, space="PSUM"))
                    yT_pool = sub.enter_context(tc.tile_pool(name="B_yT", bufs=2))
                    x2p = sub.enter_context(tc.tile_pool(name="B_x2p", bufs=1))
                    for m in range(ntt):
                        yT_m = yT_pool.tile([P, D_T, P], f32r, tag="yT_m")
                        for fidx in range(D_T):
                            tps = ps.tile([P, P], f32r, tag="tp_ps")
                            nc.tensor.transpose(tps, y[:, m, fidx * P:(fidx + 1) * P], ident)
                            nc.scalar.copy(out=yT_m[:, fidx, :], in_=tps)
                        x_t = x2p.tile([P, DIM], f32, tag="B_x2t")
                        nc.sync.dma_start(out=x_t, in_=x_own.ap()[tok0 + m * P:tok0 + (m + 1) * P, :])
                        x2_t = x2p.tile([P, DIM], f32, tag="B_x2")
                        for nh in range(2):
                            pst = ps_at.tile([P, 512], f32, tag="B_at")
                            for k in range(D_T):
                                nc.tensor.matmul(pst, yT_m[:, k, :], wo_ts[nh][:, k, :],
                                                 start=(k == 0), stop=(k == D_T - 1))
                            nc.vector.tensor_add(out=x2_t[:, nh * 512:(nh + 1) * 512],
                                                 in0=x_t[:, nh * 512:(nh + 1) * 512], in1=pst)
                        ti = (tok0 // P) + m
                        nc.scalar.dma_start(out=x2_d[ti], in_=x2_t)
                        if debug_outputs:
                            nc.sync.dma_start(out=x2_dbg.ap()[ti * P:(ti + 1) * P, :], in_=x2_t)
                        h2_r = x2p.tile([P, DIM], f32r, tag="B_h2")
                        _layernorm_tile(nc, ln_w, x2_t, g2, b2, eps_t, h2_r)
                        for fidx in range(D_T):
                            tps = ps.tile([P, P], f32r, tag="B_h2tp")
                            nc.tensor.transpose(tps, h2_r[:, fidx * P:(fidx + 1) * P], ident)
                            h2T_s = x2p.tile([P, P], f32r, tag="B_h2T")
                            nc.scalar.copy(out=h2T_s, in_=tps)
                            nc.scalar.dma_start(out=hT_a[:, fidx, ti * P:(ti + 1) * P], in_=h2T_s)

        ab_stack.close()

        # ---------------- Phase C: CMS FFN levels ----------------
        hT_io = [(hT_a, hT_b), (hT_b, hT_a), (hT_a, hT_b)]
        TT_D = 512
        n_tt_d = t_own // TT_D
        HK = H_T // 2  # half of the hidden k-tiles
        for lvl in range(LEVELS):
            hT_in, hT_out = hT_io[lvl]
            with ExitStack() as ph:
                b1s = ph.enter_context(tc.tile_pool(name=f"C{lvl}_b", bufs=1))
                b1_t = b1s.tile([P, H_T], f32)
                nc.sync.dma_start(out=b1_t, in_=cms_b1.ap()[lvl].rearrange("(m p) -> p m", p=P))
                b2_t = b1s.tile([P, D_T], f32)
                nc.sync.dma_start(out=b2_t, in_=cms_b2.ap()[lvl].rearrange("(m p) -> p m", p=P))
                # w2 pool opened before UP so its 16 MiB load overlaps up compute
                w2sb = ph.enter_context(tc.tile_pool(name=f"C{lvl}_w2", bufs=1))
                w2_sb = w2sb.tile([P, H_T, DIM], f32r)
                w2_all = cms_w2.ap()[lvl].rearrange("(kt p) d -> p kt d", p=P)
                for kh in range(2):
                    nc.scalar.dma_start(out=w2_sb[:, kh * HK:(kh + 1) * HK, :],
                                        in_=w2_all[:, kh * HK:(kh + 1) * HK, :])
                # UP: hT_in resident, w1 streamed; upg -> DRAM
                with ExitStack() as sub:
                    sb = sub.enter_context(tc.tile_pool(name=f"C{lvl}_up_sb", bufs=1))
                    wst = sub.enter_context(tc.tile_pool(name=f"C{lvl}_up_w", bufs=2))
                    ps = sub.enter_context(tc.tile_pool(name=f"C{lvl}_up_ps", bufs=8, space="PSUM"))
                    ostg = sub.enter_context(tc.tile_pool(name=f"C{lvl}_up_o", bufs=3))
                    hT_sb = sb.tile([P, D_T, t_own], f32r)
                    for k in range(D_T):
                        for tc_ in range(n_tt_up):
                            nc.sync.dma_start(out=hT_sb[:, k, tc_ * TT_UP:(tc_ + 1) * TT_UP],
                                              in_=hT_in[:, k, tc_ * TT_UP:(tc_ + 1) * TT_UP])
                    w1_all = cms_w1.ap()[lvl].rearrange("(kt p) d -> p kt d", p=P)
                    for m in range(H_T):
                        w_t = wst.tile([P, D_T, P], f32r, tag="up_w")
                        nc.sync.dma_start(out=w_t, in_=w1_all[:, :, m * P:(m + 1) * P])
                        psl = [ps.tile([P, TT_UP], f32, tag="up_ps", name=f"up_ps_{m}_{i}") for i in range(n_tt_up)]
                        for k in range(D_T):
                            for tt in range(n_tt_up):
                                nc.tensor.matmul(psl[tt], w_t[:, k, :], hT_sb[:, k, tt * TT_UP:(tt + 1) * TT_UP],
                                                 start=(k == 0), stop=(k == D_T - 1))
                        for tt in range(n_tt_up):
                            og = ostg.tile([P, TT_UP], f32r, tag="up_og")
                            nc.scalar.activation(out=og, in_=psl[tt], func=AF.Gelu_apprx_tanh,
                                                 bias=b1_t[:, m:m + 1], scale=1.0)
                            nc.sync.dma_start(out=upg_d[:, m, tt * TT_UP:(tt + 1) * TT_UP], in_=og)
                # DOWN: w2 resident; upg streamed in half-K slices (ping-pong)
                with ExitStack() as sub:
                    sb = sub.enter_context(tc.tile_pool(name=f"C{lvl}_dn_sb", bufs=2))
                    ps = sub.enter_context(tc.tile_pool(name=f"C{lvl}_dn_ps", bufs=8, space="PSUM"))
                    ostg = sub.enter_context(tc.tile_pool(name=f"C{lvl}_dn_o", bufs=3))
                    for tt in range(n_tt_d):
                        ug_h = []
                        for kh in range(2):
                            ug = sb.tile([P, HK, TT_D], f32r, tag="dn_ug", name=f"dn_ug_{lvl}_{tt}_{kh}")
                            nc.sync.dma_start(
                                out=ug, in_=upg_d[:, kh * HK:(kh + 1) * HK, tt * TT_D:(tt + 1) * TT_D])
                            ug_h.append(ug)
                        pstl = [ps.tile([P, TT_D], f32, tag="dn_ps", name=f"dn_ps_{lvl}_{tt}_{m}")
                                for m in range(D_T)]
                        for kh in range(2):
                            for m in range(D_T):
                                for k in range(HK):
                                    nc.tensor.matmul(pstl[m], w2_sb[:, kh * HK + k, m * P:(m + 1) * P],
                                                     ug_h[kh][:, k, :],
                                                     start=(kh == 0 and k == 0),
                                                     stop=(kh == 1 and k == HK - 1))
                        for m in range(D_T):
                            og = ostg.tile([P, TT_D], f32r, tag="dn_og")
                            nc.vector.tensor_scalar_add(out=og, in0=pstl[m], scalar1=b2_t[:, m:m + 1])
                            nc.sync.dma_start(out=hT_out[:, m, tt * TT_D:(tt + 1) * TT_D], in_=og)

        # ---------------- Phase D: out = x2 + h^T ----------------
        hT_fin = hT_io[LEVELS - 1][1]
        with ExitStack() as ph:
            sb = ph.enter_context(tc.tile_pool(name="D_sb", bufs=3))
            ps = ph.enter_context(tc.tile_pool(name="D_ps", bufs=3, space="PSUM"))
            for t in range(n_own_t):
                x2_t = sb.tile([P, DIM], f32, tag="D_x2")
                nc.sync.dma_start(out=x2_t, in_=x2_d[t])
                o_t = sb.tile([P, DIM], f32, tag="D_o")
                hsl = sb.tile([P, D_T, P], f32r, tag="D_h")
                nc.sync.dma_start(out=hsl, in_=hT_fin[:, :, t * P:(t + 1) * P])
                for fidx in range(D_T):
                    tps = ps.tile([P, P], f32r, tag="D_tp")
                    nc.tensor.transpose(tps, hsl[:, fidx, :], ident)
                    nc.vector.tensor_add(out=o_t[:, fidx * P:(fidx + 1) * P],
                                         in0=x2_t[:, fidx * P:(fidx + 1) * P],
                                         in1=tps.bitcast(f32))
                nc.scalar.dma_start(out=out.ap()[t * P:(t + 1) * P, :], in_=o_t)

    _split_multi_waits(nc)
    return nc


_NC_CACHE = {}
LAST_RESULT = None


def _get_nc(key, **kw):
    if key not in _NC_CACHE:
        _NC_CACHE[key] = build_kernel(**kw)
    return _NC_CACHE[key]


def kernel(x, ln1_g, ln1_b, wq, wk, wv, wo, ln2_g, ln2_b,
           cms_w1, cms_b1, cms_w2, cms_b2, **extra):
    x = np.asarray(x, np.float32)
    maskT = np.triu(np.ones((CHUNK, CHUNK), np.float32))  # maskT[e,c] = e<=c
    common = {
        "wq": np.asarray(wq, np.float32), "wk": np.asarray(wk, np.float32),
        "wv": np.asarray(wv, np.float32), "wo": np.asarray(wo, np.float32),
        "ln1_g": np.asarray(ln1_g, np.float32), "ln1_b": np.asarray(ln1_b, np.float32),
        "ln2_g": np.asarray(ln2_g, np.float32), "ln2_b": np.asarray(ln2_b, np.float32),
        "cms_w1": np.asarray(cms_w1, np.float32), "cms_b1": np.asarray(cms_b1, np.float32),
        "cms_w2": np.asarray(cms_w2, np.float32), "cms_b2": np.asarray(cms_b2, np.float32),
        "maskT": maskT,
    }
    zeros_pre = np.zeros((T_PRE, DIM), np.float32)
    in_maps = []
    for c in range(N_CORES):
        b, half = c // 2, c % 2
        own = x[b, half * T_OWN:(half + 1) * T_OWN]
        pre = x[b, 0:T_PRE] if half else zeros_pre
        in_maps.append({**common, "x_own": np.ascontiguousarray(own),
                        "x_pre": np.ascontiguousarray(pre)})
    nc = _get_nc("full")
    res = run_bass_kernel_spmd(nc, in_maps, core_ids=list(range(N_CORES)))
    global LAST_RESULT
    LAST_RESULT = res
    out = np.empty((B, S, DIM), np.float32)
    for c in range(N_CORES):
        b, half = c // 2, c % 2
        out[b, half * T_OWN:(half + 1) * T_OWN] = res.results[c]["out"]
    return out



# revision 13
# speedup vs baseline: 1.1743x; 1.1743x over previous
"""HOPE block kernel for 8 Trainium2 NeuronCores.

Sharding: 8 shards = (batch b in 0..3, sequence half in 0..1), 2048 tokens each.
The linear-attention memory scan is causal per (batch, head); cores owning the
second half of a sequence receive the first half as a "prefix" input and
rebuild the mid-sequence memory state M (= sum_t k_t v_t^T per head) before
scanning their own chunks. Even cores receive a zero prefix, keeping the SPMD
program uniform.

v2: bf16 operands for every matmul (fp32r pays 4 cycles/row below N=256, bf16
is 1 everywhere; weights/activations halve DMA + SBUF), the CMS FFN keeps h /
gelu activations / the partial-sum accumulator resident in SBUF (no DRAM
round-trips; only w1/w2 stream, once per level), persistent tile pools so
blocks/levels pipeline, and the scan loops chunk-outer/head-inner so the
per-head memory update chain is 16 iterations deep instead of 1.
"""
import sys
if '/opt/trn_rl_repo' not in sys.path:
    sys.path.insert(0, '/opt/trn_rl_repo')

from contextlib import ExitStack
import numpy as np


def _ensure_ntff_hook():
    """Register the axon NTFF profile hook when the image's antenv stub
    lacks `axon_hooks` — otherwise BASS_TRACE runs skip tracing and
    exec_time_ns comes back None. No-op when the real module exists."""
    import types, contextlib, ctypes, os
    try:
        from antenv.axon_hooks import get_axon_ntff_profile_hook  # noqa: F401
        return
    except ImportError:
        pass
    hook = None
    so_path = '/opt/axon/libaxon_pjrt.so'
    if os.path.exists(so_path):
        try:
            lib = ctypes.CDLL(so_path)
        except OSError:
            lib = None
        if lib is not None and hasattr(lib, 'axon_start_nrt_profile'):
            lib.axon_start_nrt_profile.argtypes = [
                ctypes.POINTER(ctypes.c_int64), ctypes.c_size_t]
            lib.axon_start_nrt_profile.restype = ctypes.c_int64
            lib.axon_stop_nrt_profile.argtypes = [ctypes.c_char_p]
            lib.axon_stop_nrt_profile.restype = ctypes.c_int64

            @contextlib.contextmanager
            def hook(output_dir, device_ids):
                import jax
                jax.devices()  # force PJRT init so the .so's client exists
                if device_ids:
                    ids = (ctypes.c_int64 * len(device_ids))(*device_ids)
                    rc = lib.axon_start_nrt_profile(ids, len(device_ids))
                else:
                    rc = lib.axon_start_nrt_profile(None, 0)
                if rc != 0:
                    raise RuntimeError(f"axon_start_nrt_profile rc={rc}")
                try:
                    yield
                finally:
                    n = lib.axon_stop_nrt_profile(str(output_dir).encode())
                    print(f"profile: {n} file(s) written to {output_dir}",
                          file=sys.stderr)

    try:
        import antenv
    except ImportError:
        return
    mod = types.ModuleType('antenv.axon_hooks')
    _h = hook
    mod.get_axon_ntff_profile_hook = lambda: _h
    mod.set_axon_ntff_profile_hook = lambda h: None
    sys.modules['antenv.axon_hooks'] = mod
    antenv.axon_hooks = mod


_ensure_ntff_hook()

import concourse.bass as bass
import concourse.tile as tile
from concourse import mybir
from concourse.bass_utils import run_bass_kernel_spmd
from concourse.masks import make_identity

f32 = mybir.dt.float32
bf16 = mybir.dt.bfloat16
AF = mybir.ActivationFunctionType
ALU = mybir.AluOpType

DIM = 1024
HEADS = 16
HD = 64
B, S = 4, 4096
LEVELS = 3
HID = 4 * DIM
CHUNK = 128
EPS = 1e-5
P = 128

N_CORES = 8
T_OWN = S // 2      # tokens per core
T_PRE = S // 2      # prefix tokens (zeros on even cores)
BLK = 512           # token block for phases A and B
D_T = DIM // P      # 8 feature tiles
H_T = HID // P      # 32 hidden tiles
HHT = H_T // 2      # 16 hidden tiles per half
TT = 512            # FFN token tile

MAX_WAITS = 1


def _split_multi_waits(nc, max_waits=MAX_WAITS):
    """Walrus in this toolchain encodes at most `max_waits` sem waits per
    instruction; split extra waits onto same-engine NOPs placed just before."""
    for f in nc.m.functions:
        for bb in f.blocks:
            insts = list(bb.instructions)
            if not any(
                i.sync_info and i.sync_info.on_wait and len(i.sync_info.on_wait) > max_waits
                for i in insts
            ):
                continue
            new = []
            for inst in insts:
                si = inst.sync_info
                waits = list(si.on_wait) if si and si.on_wait else []
                if len(waits) > max_waits:
                    head, rest = waits[:-max_waits], waits[-max_waits:]
                    while head:
                        chunk, head = head[:max_waits], head[max_waits:]
                        nop = mybir.InstNoOp(name=nc.get_next_instruction_name(), ins=[], outs=[])
                        nop.engine = inst.engine
                        nop.sync_info = mybir.SyncInfo(on_wait=chunk, on_update=[])
                        nc.register_instruction(nop, overwrite=True)
                        new.append(nop)
                    inst.sync_info = mybir.SyncInfo(
                        on_wait=rest, on_update=list(si.on_update) if si.on_update else [])
                new.append(inst)
            bb.instructions = new


def _layernorm_tile(nc, pools, x_t, g_bc, b_bc, eps_t, out_r):
    """LayerNorm of one [128, DIM] fp32 tile -> bf16 tile (token-major)."""
    w = pools
    BNF = nc.vector.BN_STATS_FMAX
    nsub = DIM // BNF
    stats = w.tile([P, nsub, nc.vector.BN_STATS_DIM], f32, tag="ln_stats")
    xg = x_t[:].rearrange("p (s f) -> p s f", f=BNF)
    for s_ in range(nsub):
        nc.vector.bn_stats(out=stats[:, s_, :], in_=xg[:, s_, :])
    mv = w.tile([P, nc.vector.BN_AGGR_DIM], f32, tag="ln_mv")
    nc.vector.bn_aggr(out=mv, in_=stats)
    rstd = w.tile([P, 1], f32, tag="ln_rstd")
    nc.scalar.activation(out=rstd, in_=mv[:, 1:2], func=AF.Sqrt, bias=eps_t, scale=1.0)
    nc.vector.reciprocal(out=rstd, in_=rstd)
    tmp = w.tile([P, DIM], f32, tag="ln_tmp")
    nc.vector.tensor_scalar(out=tmp, in0=x_t, scalar1=mv[:, 0:1], scalar2=rstd,
                            op0=ALU.subtract, op1=ALU.mult)
    nc.vector.tensor_mul(out=tmp, in0=tmp, in1=g_bc)
    nc.vector.tensor_add(out=out_r, in0=tmp, in1=b_bc)


def build_kernel(t_own=T_OWN, t_pre=T_PRE):
    nc = bass.Bass()

    x_own = nc.dram_tensor("x_own", [t_own, DIM], f32, kind="ExternalInput")
    x_pre = nc.dram_tensor("x_pre", [t_pre, DIM], f32, kind="ExternalInput")
    wq = nc.dram_tensor("wq", [DIM, DIM], bf16, kind="ExternalInput")
    wk = nc.dram_tensor("wk", [DIM, DIM], bf16, kind="ExternalInput")
    wv = nc.dram_tensor("wv", [DIM, DIM], bf16, kind="ExternalInput")
    wo = nc.dram_tensor("wo", [DIM, DIM], bf16, kind="ExternalInput")
    ln1_g = nc.dram_tensor("ln1_g", [DIM], f32, kind="ExternalInput")
    ln1_b = nc.dram_tensor("ln1_b", [DIM], f32, kind="ExternalInput")
    ln2_g = nc.dram_tensor("ln2_g", [DIM], f32, kind="ExternalInput")
    ln2_b = nc.dram_tensor("ln2_b", [DIM], f32, kind="ExternalInput")
    cms_w1 = nc.dram_tensor("cms_w1", [LEVELS, DIM, HID], bf16, kind="ExternalInput")
    cms_b1 = nc.dram_tensor("cms_b1", [LEVELS, HID], f32, kind="ExternalInput")
    cms_w2 = nc.dram_tensor("cms_w2", [LEVELS, HID, DIM], bf16, kind="ExternalInput")
    cms_b2 = nc.dram_tensor("cms_b2", [LEVELS, DIM], f32, kind="ExternalInput")
    maskT = nc.dram_tensor("maskT", [CHUNK, CHUNK], f32, kind="ExternalInput")
    out = nc.dram_tensor("out", [t_own, DIM], f32, kind="ExternalOutput")

    n_own_t = t_own // P           # 16 token tiles of 128
    n_blk = t_own // BLK           # 4 blocks
    n_pre_blk = t_pre // BLK       # 4 prefix blocks
    ntt = BLK // P                 # 4 token tiles per block
    n_tt = t_own // TT             # 4 FFN token tiles

    with tile.TileContext(nc) as tc, ExitStack() as top:
        consts = top.enter_context(tc.tile_pool(name="consts", bufs=1))
        ident_f = consts.tile([P, P], f32)
        make_identity(nc, ident_f)
        ident = consts.tile([P, P], bf16)
        nc.vector.tensor_copy(out=ident, in_=ident_f)
        eps_t = consts.tile([P, 1], f32)
        nc.vector.memset(eps_t, EPS)
        mask_t = consts.tile([CHUNK, CHUNK], f32)
        nc.sync.dma_start(out=mask_t, in_=maskT.ap())
        g1 = consts.tile([P, DIM], f32)
        b1 = consts.tile([P, DIM], f32)
        g2 = consts.tile([P, DIM], f32)
        b2 = consts.tile([P, DIM], f32)
        nc.sync.dma_start(out=g1, in_=ln1_g.ap()[None, :].partition_broadcast(P).opt())
        nc.sync.dma_start(out=b1, in_=ln1_b.ap()[None, :].partition_broadcast(P).opt())
        nc.sync.dma_start(out=g2, in_=ln2_g.ap()[None, :].partition_broadcast(P).opt())
        nc.sync.dma_start(out=b2, in_=ln2_b.ap()[None, :].partition_broadcast(P).opt())

        # persistent activation: h2^T (feature-major, FFN input/output),
        # bf16, SBUF-resident to the end; x2 residual spills to DRAM in bf16
        persist = top.enter_context(tc.tile_pool(name="persist", bufs=1))
        hT = persist.tile([P, D_T, t_own], bf16)
        dram = top.enter_context(tc.tile_pool(name="dram", bufs=1, space="DRAM"))
        x2_d = dram.tile([n_own_t, P, DIM], bf16)

        # ---------------- attention phases ----------------
        ab_stack = ExitStack()
        wo_pool = ab_stack.enter_context(tc.tile_pool(name="wo_pool", bufs=1))
        wo_sb = wo_pool.tile([P, D_T, DIM], bf16)
        wo_all = wo.ap().rearrange("(kt p) d -> p kt d", p=P)
        nc.scalar.dma_start(out=wo_sb, in_=wo_all)

        mt_pool = ab_stack.enter_context(tc.tile_pool(name="mt", bufs=1))
        Mt_f = mt_pool.tile([P, HEADS // 2, HD], f32)   # head h at [pb:pb+64, h//2]
        Mt_s = mt_pool.tile([P, HEADS // 2, HD], bf16)
        nc.vector.memset(Mt_f, 0.0)

        ln_w = ab_stack.enter_context(tc.tile_pool(name="ln_w", bufs=2))
        xp = ab_stack.enter_context(tc.tile_pool(name="xp", bufs=2))
        hrp = ab_stack.enter_context(tc.tile_pool(name="hrp", bufs=2))
        h1Tp = ab_stack.enter_context(tc.tile_pool(name="h1Tp", bufs=2))
        wsp = ab_stack.enter_context(tc.tile_pool(name="wsp", bufs=2))
        actp = ab_stack.enter_context(tc.tile_pool(name="actp", bufs=1))
        ps_tp = ab_stack.enter_context(tc.tile_pool(name="ps_tp", bufs=2, space="PSUM"))
        ps_mm = ab_stack.enter_context(tc.tile_pool(name="ps_mm", bufs=2, space="PSUM"))
        ps_scan = ab_stack.enter_context(tc.tile_pool(name="ps_scan", bufs=1, space="PSUM"))

        def ln_transpose_block(x_src, tok0, g_bc, b_bc, dstT):
            """DMA 4 x-tiles, LayerNorm, PE-transpose into dstT [P, D_T, BLK].
            Returns the list of x tiles (fp32) for residual use."""
            xts = []
            for t in range(ntt):
                x_t = xp.tile([P, DIM], f32, tag=f"x{t % 2}")
                nc.sync.dma_start(out=x_t, in_=x_src.ap()[tok0 + t * P:tok0 + (t + 1) * P, :])
                h_r = hrp.tile([P, DIM], bf16, tag=f"h{t % 2}")
                _layernorm_tile(nc, ln_w, x_t, g_bc, b_bc, eps_t, h_r)
                for fidx in range(D_T):
                    tps = ps_tp.tile([P, P], bf16, tag="tp_ps")
                    nc.tensor.transpose(tps, h_r[:, fidx * P:(fidx + 1) * P], ident)
                    nc.scalar.copy(out=dstT[:, fidx, t * P:(t + 1) * P], in_=tps)
                xts.append(x_t)
            return xts

        # ---------------- Phase A: prefix -> Mt ----------------
        for blk in range(n_pre_blk):
            hpT = h1Tp.tile([P, D_T, BLK], bf16, tag="h1T")
            ln_transpose_block(x_pre, blk * BLK, g1, b1, hpT)
            kc = actp.tile([P, ntt, DIM], bf16, tag="kc")
            vc = actp.tile([P, ntt, DIM], bf16, tag="vc")
            for (w_in, dst) in ((wk, kc), (wv, vc)):
                w_all = w_in.ap().rearrange("(kt p) d -> p kt d", p=P)
                for nh in range(2):
                    w_t = wsp.tile([P, D_T, 512], bf16, tag="w_t")
                    nc.sync.dma_start(out=w_t, in_=w_all[:, :, nh * 512:(nh + 1) * 512])
                    for m in range(ntt):
                        pst = ps_mm.tile([P, 512], f32, tag="pst")
                        for k in range(D_T):
                            nc.tensor.matmul(pst, hpT[:, k, m * P:(m + 1) * P], w_t[:, k, :],
                                             start=(k == 0), stop=(k == D_T - 1))
                        nc.scalar.copy(out=dst[:, m, nh * 512:(nh + 1) * 512], in_=pst)
            # Mt += kc^T vc per head, accumulated in PSUM across the block
            for h in range(HEADS):
                pb = (h % 2) * HD
                mt_ps = ps_scan.tile([HD, HD], f32, tag="sc" if h % 2 == 0 else "y")
                for ch in range(ntt):
                    nc.tensor.matmul(mt_ps, kc[:, ch, h * HD:(h + 1) * HD],
                                     vc[:, ch, h * HD:(h + 1) * HD],
                                     start=(ch == 0), stop=(ch == ntt - 1))
                nc.vector.tensor_add(out=Mt_f[pb:pb + HD, h // 2, :],
                                     in0=Mt_f[pb:pb + HD, h // 2, :], in1=mt_ps)
        nc.scalar.copy(out=Mt_s, in_=Mt_f)

        # ---------------- Phase B: own tokens, attention ----------------
        scw = ab_stack.enter_context(tc.tile_pool(name="scw", bufs=3))
        x2fp = ab_stack.enter_context(tc.tile_pool(name="x2fp", bufs=2))
        yTp = ab_stack.enter_context(tc.tile_pool(name="yTp", bufs=2))
        for blk in range(n_blk):
            tok0 = blk * BLK
            h1T = h1Tp.tile([P, D_T, BLK], bf16, tag="h1T")
            ln_transpose_block(x_own, tok0, g1, b1, h1T)
            qT = actp.tile([P, D_T, BLK], bf16, tag="qT")
            kT = actp.tile([P, D_T, BLK], bf16, tag="kT")
            for (w_in, dst) in ((wq, qT), (wk, kT)):
                w_all = w_in.ap().rearrange("(kt p) d -> p kt d", p=P)
                for nh in range(2):
                    w_t = wsp.tile([P, D_T, 512], bf16, tag="w_t")
                    nc.sync.dma_start(out=w_t, in_=w_all[:, :, nh * 512:(nh + 1) * 512])
                    for ml in range(4):
                        m = nh * 4 + ml
                        pst = ps_mm.tile([P, BLK], f32, tag="pst")
                        for k in range(D_T):
                            nc.tensor.matmul(pst, w_t[:, k, ml * P:(ml + 1) * P], h1T[:, k, :],
                                             start=(k == 0), stop=(k == D_T - 1))
                        nc.scalar.copy(out=dst[:, m, :], in_=pst)
            v = actp.tile([P, ntt, DIM], bf16, tag="vc")
            wv_all = wv.ap().rearrange("(kt p) d -> p kt d", p=P)
            for nh in range(2):
                w_t = wsp.tile([P, D_T, 512], bf16, tag="w_t")
                nc.sync.dma_start(out=w_t, in_=wv_all[:, :, nh * 512:(nh + 1) * 512])
                for m in range(ntt):
                    pst = ps_mm.tile([P, 512], f32, tag="pst")
                    for k in range(D_T):
                        nc.tensor.matmul(pst, h1T[:, k, m * P:(m + 1) * P], w_t[:, k, :],
                                         start=(k == 0), stop=(k == D_T - 1))
                    nc.scalar.copy(out=v[:, m, nh * 512:(nh + 1) * 512], in_=pst)
            # scan: chunk-outer / head-inner so Mt chains are 16 apart
            y = actp.tile([P, ntt, DIM], bf16, tag="y")
            for ch in range(ntt):
                for h in range(HEADS):
                    pb = (h % 2) * HD
                    fi = h // 2
                    qcT = qT[pb:pb + HD, fi, ch * P:(ch + 1) * P]
                    kcT = kT[pb:pb + HD, fi, ch * P:(ch + 1) * P]
                    vc_s = v[:, ch, h * HD:(h + 1) * HD]
                    kc_ps = ps_scan.tile([P, HD], bf16, tag="kc")
                    nc.tensor.transpose(kc_ps, kcT, ident[pb:pb + HD, pb:pb + HD])
                    kc_s = scw.tile([P, HD], bf16, tag="kc_s")
                    nc.scalar.copy(out=kc_s, in_=kc_ps)
                    sc_ps = ps_scan.tile([P, P], f32, tag="sc")
                    nc.tensor.matmul(sc_ps, kcT, qcT, start=True, stop=True)
                    sc_r = scw.tile([P, P], bf16, tag="sc_r")
                    nc.vector.tensor_mul(out=sc_r, in0=sc_ps, in1=mask_t)
                    y_ps = ps_scan.tile([P, HD], f32, tag="y")
                    nc.tensor.matmul(y_ps, sc_r, vc_s, start=True, stop=False)
                    nc.tensor.matmul(y_ps, qcT, Mt_s[pb:pb + HD, fi, :], start=False, stop=True)
                    nc.scalar.copy(out=y[:, ch, h * HD:(h + 1) * HD], in_=y_ps)
                    mt_ps = ps_scan.tile([HD, HD], f32, tag="mt")
                    nc.tensor.matmul(mt_ps, kc_s, vc_s, start=True, stop=True)
                    nc.vector.tensor_add(out=Mt_f[pb:pb + HD, fi, :],
                                         in0=Mt_f[pb:pb + HD, fi, :], in1=mt_ps)
                    nc.scalar.copy(out=Mt_s[pb:pb + HD, fi, :], in_=Mt_f[pb:pb + HD, fi, :])
            # epilogue: yT, attn-out + residual, LN2, h2T into persistent hT
            for m in range(ntt):
                yT_m = yTp.tile([P, D_T, P], bf16, tag="yT_m")
                for fidx in range(D_T):
                    tps = ps_tp.tile([P, P], bf16, tag="tp_ps")
                    nc.tensor.transpose(tps, y[:, m, fidx * P:(fidx + 1) * P], ident)
                    nc.scalar.copy(out=yT_m[:, fidx, :], in_=tps)
                x_t = xp.tile([P, DIM], f32, tag=f"x{m % 2}")
                nc.sync.dma_start(out=x_t, in_=x_own.ap()[tok0 + m * P:tok0 + (m + 1) * P, :])
                ti = (tok0 // P) + m
                x2f = x2fp.tile([P, DIM], f32, tag="x2f")
                for nh in range(2):
                    pst = ps_mm.tile([P, 512], f32, tag="pst")
                    for k in range(D_T):
                        nc.tensor.matmul(pst, yT_m[:, k, :], wo_sb[:, k, nh * 512:(nh + 1) * 512],
                                         start=(k == 0), stop=(k == D_T - 1))
                    nc.vector.tensor_add(out=x2f[:, nh * 512:(nh + 1) * 512],
                                         in0=x_t[:, nh * 512:(nh + 1) * 512], in1=pst)
                x2b = x2fp.tile([P, DIM], bf16, tag="x2b")
                nc.scalar.copy(out=x2b, in_=x2f)
                nc.scalar.dma_start(out=x2_d[ti], in_=x2b)
                h2_r = hrp.tile([P, DIM], bf16, tag="h2r")
                _layernorm_tile(nc, ln_w, x2f, g2, b2, eps_t, h2_r)
                for fidx in range(D_T):
                    tps = ps_tp.tile([P, P], bf16, tag="tp_ps")
                    nc.tensor.transpose(tps, h2_r[:, fidx * P:(fidx + 1) * P], ident)
                    nc.scalar.copy(out=hT[:, fidx, ti * P:(ti + 1) * P], in_=tps)

        ab_stack.close()

        # ---------------- Phase C: CMS FFN, fully SBUF-resident ----------------
        with ExitStack() as ffn:
            big = ffn.enter_context(tc.tile_pool(name="ffn_big", bufs=1))
            upg = big.tile([P, HHT, t_own], bf16)       # gelu acts, one hidden half
            out_acc = big.tile([P, D_T, t_own], bf16)   # half-0 partial sums + b2
            bp = ffn.enter_context(tc.tile_pool(name="ffn_b", bufs=2))
            w1s = ffn.enter_context(tc.tile_pool(name="w1s", bufs=2))
            w2s = ffn.enter_context(tc.tile_pool(name="w2s", bufs=2))
            ps_up = ffn.enter_context(tc.tile_pool(name="ps_up", bufs=4, space="PSUM"))
            ps_dn = ffn.enter_context(tc.tile_pool(name="ps_dn", bufs=4, space="PSUM"))
            for lvl in range(LEVELS):
                b1_t = bp.tile([P, H_T], f32, tag="b1")
                nc.sync.dma_start(out=b1_t, in_=cms_b1.ap()[lvl].rearrange("(m p) -> p m", p=P))
                b2_t = bp.tile([P, D_T], f32, tag="b2")
                nc.sync.dma_start(out=b2_t, in_=cms_b2.ap()[lvl].rearrange("(m p) -> p m", p=P))
                w1_all = cms_w1.ap()[lvl].rearrange("(kt p) d -> p kt d", p=P)
                w2_all = cms_w2.ap()[lvl].rearrange("(kt p) d -> p kt d", p=P)
                for half in range(2):
                    for mg in range(4):  # 512 hidden cols per w1 tile
                        w1_t = w1s.tile([P, D_T, 512], bf16, tag="w1t")
                        c0 = half * (HID // 2) + mg * 512
                        nc.sync.dma_start(out=w1_t, in_=w1_all[:, :, c0:c0 + 512])
                        for ml in range(4):
                            mh = mg * 4 + ml           # hidden tile within half
                            m_gl = half * HHT + mh     # global hidden tile
                            for tt in range(n_tt):
                                ps = ps_up.tile([P, TT], f32, tag="up")
                                for k in range(D_T):
                                    nc.tensor.matmul(ps, w1_t[:, k, ml * P:(ml + 1) * P],
                                                     hT[:, k, tt * TT:(tt + 1) * TT],
                                                     start=(k == 0), stop=(k == D_T - 1))
                                nc.scalar.activation(
                                    out=upg[:, mh, tt * TT:(tt + 1) * TT], in_=ps,
                                    func=AF.Gelu_apprx_tanh,
                                    bias=b1_t[:, m_gl:m_gl + 1], scale=1.0)
                    for mdg in range(4):  # 256 output cols per w2 tile
                        w2_t = w2s.tile([P, HHT, 256], bf16, tag="w2t")
                        nc.sync.dma_start(
                            out=w2_t,
                            in_=w2_all[:, half * HHT:(half + 1) * HHT, mdg * 256:(mdg + 1) * 256])
                        for mdl in range(2):
                            md = mdg * 2 + mdl
                            for tt in range(n_tt):
                                ps = ps_dn.tile([P, TT], f32, tag="dn")
                                for k in range(HHT):
                                    nc.tensor.matmul(ps, w2_t[:, k, mdl * P:(mdl + 1) * P],
                                                     upg[:, k, tt * TT:(tt + 1) * TT],
                                                     start=(k == 0), stop=(k == HHT - 1))
                                if half == 0:
                                    nc.scalar.activation(
                                        out=out_acc[:, md, tt * TT:(tt + 1) * TT], in_=ps,
                                        func=AF.Identity, bias=b2_t[:, md:md + 1], scale=1.0)
                                else:
                                    nc.vector.tensor_add(
                                        out=hT[:, md, tt * TT:(tt + 1) * TT],
                                        in0=out_acc[:, md, tt * TT:(tt + 1) * TT], in1=ps)

        # ---------------- Phase D: out = x2 + h^T ----------------
        with ExitStack() as ph:
            sb = ph.enter_context(tc.tile_pool(name="D_sb", bufs=3))
            ps = ph.enter_context(tc.tile_pool(name="D_ps", bufs=3, space="PSUM"))
            for t in range(n_own_t):
                x2_t = sb.tile([P, DIM], bf16, tag="D_x2")
                nc.sync.dma_start(out=x2_t, in_=x2_d[t])
                o_t = sb.tile([P, DIM], f32, tag="D_o")
                for fidx in range(D_T):
                    tps = ps.tile([P, P], bf16, tag="D_tp")
                    nc.tensor.transpose(tps, hT[:, fidx, t * P:(t + 1) * P], ident)
                    nc.vector.tensor_add(out=o_t[:, fidx * P:(fidx + 1) * P],
                                         in0=x2_t[:, fidx * P:(fidx + 1) * P],
                                         in1=tps)
                nc.scalar.dma_start(out=out.ap()[t * P:(t + 1) * P, :], in_=o_t)

    _split_multi_waits(nc)
    return nc


_NC_CACHE = {}
LAST_RESULT = None


def _get_nc(key, **kw):
    if key not in _NC_CACHE:
        _NC_CACHE[key] = build_kernel(**kw)
    return _NC_CACHE[key]


def kernel(x, ln1_g, ln1_b, wq, wk, wv, wo, ln2_g, ln2_b,
           cms_w1, cms_b1, cms_w2, cms_b2, **extra):
    import ml_dtypes
    bf = ml_dtypes.bfloat16
    x = np.asarray(x, np.float32)
    maskT = np.triu(np.ones((CHUNK, CHUNK), np.float32))  # maskT[e,c] = e<=c
    common = {
        "wq": np.asarray(wq, bf), "wk": np.asarray(wk, bf),
        "wv": np.asarray(wv, bf), "wo": np.asarray(wo, bf),
        "ln1_g": np.asarray(ln1_g, np.float32), "ln1_b": np.asarray(ln1_b, np.float32),
        "ln2_g": np.asarray(ln2_g, np.float32), "ln2_b": np.asarray(ln2_b, np.float32),
        "cms_w1": np.asarray(cms_w1, bf), "cms_b1": np.asarray(cms_b1, np.float32),
        "cms_w2": np.asarray(cms_w2, bf), "cms_b2": np.asarray(cms_b2, np.float32),
        "maskT": maskT,
    }
    zeros_pre = np.zeros((T_PRE, DIM), np.float32)
    in_maps = []
    for c in range(N_CORES):
        b, half = c // 2, c % 2
        own = x[b, half * T_OWN:(half + 1) * T_OWN]
        pre = x[b, 0:T_PRE] if half else zeros_pre
        in_maps.append({**common, "x_own": np.ascontiguousarray(own),
                        "x_pre": np.ascontiguousarray(pre)})
    nc = _get_nc("full")
    res = run_bass_kernel_spmd(nc, in_maps, core_ids=list(range(N_CORES)))
    global LAST_RESULT
    LAST_RESULT = res
    out = np.empty((B, S, DIM), np.float32)
    for c in range(N_CORES):
        b, half = c // 2, c % 2
        out[b, half * T_OWN:(half + 1) * T_OWN] = res.results[c]["out"]
    return out


# revision 32
# speedup vs baseline: 1.9550x; 1.6648x over previous
"""HOPE block kernel for 8 Trainium2 NeuronCores.

Sharding: 8 shards = (batch b in 0..3, sequence half in 0..1), 2048 tokens each.
The linear-attention memory scan is causal per (batch, head); cores owning the
second half of a sequence receive the first half as a "prefix" input and
rebuild the mid-sequence memory state M (= sum_t k_t v_t^T per head) before
scanning their own chunks. Even cores receive a zero prefix, keeping the SPMD
program uniform.

v2: bf16 operands for every matmul (fp32r pays 4 cycles/row below N=256, bf16
is 1 everywhere; weights/activations halve DMA + SBUF), the CMS FFN keeps h /
gelu activations / the partial-sum accumulator resident in SBUF (no DRAM
round-trips; only w1/w2 stream, once per level), persistent tile pools so
blocks/levels pipeline, and the scan loops chunk-outer/head-inner so the
per-head memory update chain is 16 iterations deep instead of 1.
"""
import sys
if '/opt/trn_rl_repo' not in sys.path:
    sys.path.insert(0, '/opt/trn_rl_repo')

from contextlib import ExitStack
import numpy as np


def _ensure_ntff_hook():
    """Register the axon NTFF profile hook when the image's antenv stub
    lacks `axon_hooks` — otherwise BASS_TRACE runs skip tracing and
    exec_time_ns comes back None. No-op when the real module exists."""
    import types, contextlib, ctypes, os
    try:
        from antenv.axon_hooks import get_axon_ntff_profile_hook  # noqa: F401
        return
    except ImportError:
        pass
    hook = None
    so_path = '/opt/axon/libaxon_pjrt.so'
    if os.path.exists(so_path):
        try:
            lib = ctypes.CDLL(so_path)
        except OSError:
            lib = None
        if lib is not None and hasattr(lib, 'axon_start_nrt_profile'):
            lib.axon_start_nrt_profile.argtypes = [
                ctypes.POINTER(ctypes.c_int64), ctypes.c_size_t]
            lib.axon_start_nrt_profile.restype = ctypes.c_int64
            lib.axon_stop_nrt_profile.argtypes = [ctypes.c_char_p]
            lib.axon_stop_nrt_profile.restype = ctypes.c_int64

            @contextlib.contextmanager
            def hook(output_dir, device_ids):
                import jax
                jax.devices()  # force PJRT init so the .so's client exists
                if device_ids:
                    ids = (ctypes.c_int64 * len(device_ids))(*device_ids)
                    rc = lib.axon_start_nrt_profile(ids, len(device_ids))
                else:
                    rc = lib.axon_start_nrt_profile(None, 0)
                if rc != 0:
                    raise RuntimeError(f"axon_start_nrt_profile rc={rc}")
                try:
                    yield
                finally:
                    n = lib.axon_stop_nrt_profile(str(output_dir).encode())
                    print(f"profile: {n} file(s) written to {output_dir}",
                          file=sys.stderr)

    try:
        import antenv
    except ImportError:
        return
    mod = types.ModuleType('antenv.axon_hooks')
    _h = hook
    mod.get_axon_ntff_profile_hook = lambda: _h
    mod.set_axon_ntff_profile_hook = lambda h: None
    sys.modules['antenv.axon_hooks'] = mod
    antenv.axon_hooks = mod


_ensure_ntff_hook()

import concourse.bass as bass
import concourse.tile as tile
from concourse import mybir
from concourse.bass_utils import run_bass_kernel_spmd
from concourse.masks import make_identity

f32 = mybir.dt.float32
bf16 = mybir.dt.bfloat16
f8 = mybir.dt.float8e4
AF = mybir.ActivationFunctionType
ALU = mybir.AluOpType
PM = mybir.MatmulPerfMode

W8_SCALE = 64.0     # host-side multiplier on fp8 FFN weights (clears the
INV_W8 = 1.0 / W8_SCALE  # e4m3 subnormal zone); un-done via activation scale

DIM = 1024
HEADS = 16
HD = 64
B, S = 4, 4096
LEVELS = 3
HID = 4 * DIM
CHUNK = 128
EPS = 1e-5
P = 128

N_CORES = 8
T_OWN = S // 2      # tokens per core
T_PRE = S // 2      # prefix tokens (zeros on even cores)
BLK = 512           # token block for phases A and B
D_T = DIM // P      # 8 feature tiles
H_T = HID // P      # 32 hidden tiles
HHT = H_T // 2      # 16 hidden tiles per half
TT = 512            # FFN token tile

MAX_WAITS = 1


def _split_multi_waits(nc, max_waits=MAX_WAITS):
    """Walrus in this toolchain encodes at most `max_waits` sem waits per
    instruction; split extra waits onto same-engine NOPs placed just before."""
    for f in nc.m.functions:
        for bb in f.blocks:
            insts = list(bb.instructions)
            if not any(
                i.sync_info and i.sync_info.on_wait and len(i.sync_info.on_wait) > max_waits
                for i in insts
            ):
                continue
            new = []
            for inst in insts:
                si = inst.sync_info
                waits = list(si.on_wait) if si and si.on_wait else []
                if len(waits) > max_waits:
                    head, rest = waits[:-max_waits], waits[-max_waits:]
                    while head:
                        chunk, head = head[:max_waits], head[max_waits:]
                        nop = mybir.InstNoOp(name=nc.get_next_instruction_name(), ins=[], outs=[])
                        nop.engine = inst.engine
                        nop.sync_info = mybir.SyncInfo(on_wait=chunk, on_update=[])
                        nc.register_instruction(nop, overwrite=True)
                        new.append(nop)
                    inst.sync_info = mybir.SyncInfo(
                        on_wait=rest, on_update=list(si.on_update) if si.on_update else [])
                new.append(inst)
            bb.instructions = new


def _layernorm_tile(nc, pools, x_t, g_bc, b_bc, eps_t, out_r):
    """LayerNorm of one [128, DIM] fp32 tile -> bf16 tile (token-major)."""
    w = pools
    BNF = nc.vector.BN_STATS_FMAX
    nsub = DIM // BNF
    stats = w.tile([P, nsub, nc.vector.BN_STATS_DIM], f32, tag="ln_stats")
    xg = x_t[:].rearrange("p (s f) -> p s f", f=BNF)
    for s_ in range(nsub):
        nc.vector.bn_stats(out=stats[:, s_, :], in_=xg[:, s_, :])
    mv = w.tile([P, nc.vector.BN_AGGR_DIM], f32, tag="ln_mv")
    nc.vector.bn_aggr(out=mv, in_=stats)
    rstd = w.tile([P, 1], f32, tag="ln_rstd")
    nc.scalar.activation(out=rstd, in_=mv[:, 1:2], func=AF.Sqrt, bias=eps_t, scale=1.0)
    nc.vector.reciprocal(out=rstd, in_=rstd)
    tmp = w.tile([P, DIM], f32, tag="ln_tmp")
    nc.vector.tensor_scalar(out=tmp, in0=x_t, scalar1=mv[:, 0:1], scalar2=rstd,
                            op0=ALU.subtract, op1=ALU.mult)
    nc.vector.tensor_mul(out=tmp, in0=tmp, in1=g_bc)
    nc.vector.tensor_add(out=out_r, in0=tmp, in1=b_bc)


def build_kernel(t_own=T_OWN, t_pre=T_PRE):
    nc = bass.Bass()

    x_own = nc.dram_tensor("x_own", [t_own, DIM], f32, kind="ExternalInput")
    x_pre = nc.dram_tensor("x_pre", [t_pre, DIM], f32, kind="ExternalInput")
    wq = nc.dram_tensor("wq", [DIM, DIM], bf16, kind="ExternalInput")
    wk = nc.dram_tensor("wk", [DIM, DIM], bf16, kind="ExternalInput")
    wv = nc.dram_tensor("wv", [DIM, DIM], bf16, kind="ExternalInput")
    wo = nc.dram_tensor("wo", [DIM, DIM], bf16, kind="ExternalInput")
    ln1_g = nc.dram_tensor("ln1_g", [DIM], f32, kind="ExternalInput")
    ln1_b = nc.dram_tensor("ln1_b", [DIM], f32, kind="ExternalInput")
    ln2_g = nc.dram_tensor("ln2_g", [DIM], f32, kind="ExternalInput")
    ln2_b = nc.dram_tensor("ln2_b", [DIM], f32, kind="ExternalInput")
    cms_w1 = nc.dram_tensor("cms_w1", [LEVELS, DIM, HID], f8, kind="ExternalInput")
    cms_b1 = nc.dram_tensor("cms_b1", [LEVELS, HID], f32, kind="ExternalInput")
    cms_w2 = nc.dram_tensor("cms_w2", [LEVELS, HID, DIM], f8, kind="ExternalInput")
    cms_b2 = nc.dram_tensor("cms_b2", [LEVELS, DIM], f32, kind="ExternalInput")
    maskT = nc.dram_tensor("maskT", [CHUNK, CHUNK], f32, kind="ExternalInput")
    out = nc.dram_tensor("out", [t_own, DIM], f32, kind="ExternalOutput")

    n_own_t = t_own // P           # 16 token tiles of 128
    n_blk = t_own // BLK           # 4 blocks
    n_pre_blk = t_pre // BLK       # 4 prefix blocks
    ntt = BLK // P                 # 4 token tiles per block
    n_tt = t_own // TT             # 4 FFN token tiles

    with tile.TileContext(nc) as tc, ExitStack() as top:
        consts = top.enter_context(tc.tile_pool(name="consts", bufs=1))
        ident_f = consts.tile([P, P], f32)
        make_identity(nc, ident_f)
        ident = consts.tile([P, P], bf16)
        nc.vector.tensor_copy(out=ident, in_=ident_f)

        eps_t = consts.tile([P, 1], f32)
        nc.vector.memset(eps_t, EPS)
        mask_t = consts.tile([CHUNK, CHUNK], f32)
        nc.sync.dma_start(out=mask_t, in_=maskT.ap())
        g1 = consts.tile([P, DIM], f32)
        b1 = consts.tile([P, DIM], f32)
        g2 = consts.tile([P, DIM], f32)
        b2 = consts.tile([P, DIM], f32)
        nc.sync.dma_start(out=g1, in_=ln1_g.ap()[None, :].partition_broadcast(P).opt())
        nc.sync.dma_start(out=b1, in_=ln1_b.ap()[None, :].partition_broadcast(P).opt())
        nc.sync.dma_start(out=g2, in_=ln2_g.ap()[None, :].partition_broadcast(P).opt())
        nc.sync.dma_start(out=b2, in_=ln2_b.ap()[None, :].partition_broadcast(P).opt())

        # persistent activation: h2^T (feature-major, FFN input/output),
        # fp8 so it feeds DoubleRow matmuls; x2 residual spills to DRAM bf16
        persist = top.enter_context(tc.tile_pool(name="persist", bufs=1))
        hT = persist.tile([P, D_T, t_own], f8)
        dram = top.enter_context(tc.tile_pool(name="dram", bufs=1, space="DRAM"))
        x2_d = dram.tile([n_own_t, P, DIM], bf16)

        # ---------------- attention phases ----------------
        ab_stack = ExitStack()
        wo_pool = ab_stack.enter_context(tc.tile_pool(name="wo_pool", bufs=1))
        wo_sb = wo_pool.tile([P, D_T, DIM], bf16)
        wo_all = wo.ap().rearrange("(kt p) d -> p kt d", p=P)
        nc.scalar.dma_start(out=wo_sb, in_=wo_all)

        mt_pool = ab_stack.enter_context(tc.tile_pool(name="mt", bufs=1))
        Mt_f = mt_pool.tile([P, HEADS // 2, HD], f32)   # head h at [pb:pb+64, h//2]
        Mt_s = mt_pool.tile([P, HEADS // 2, HD], bf16)
        nc.vector.memset(Mt_f, 0.0)

        ln_w = ab_stack.enter_context(tc.tile_pool(name="ln_w", bufs=2))
        xp = ab_stack.enter_context(tc.tile_pool(name="xp", bufs=2))
        hrp = ab_stack.enter_context(tc.tile_pool(name="hrp", bufs=2))
        h1Tp = ab_stack.enter_context(tc.tile_pool(name="h1Tp", bufs=2))
        wsp = ab_stack.enter_context(tc.tile_pool(name="wsp", bufs=2))
        actp = ab_stack.enter_context(tc.tile_pool(name="actp", bufs=1))
        ps_tp = ab_stack.enter_context(tc.tile_pool(name="ps_tp", bufs=2, space="PSUM"))
        ps_mm = ab_stack.enter_context(tc.tile_pool(name="ps_mm", bufs=2, space="PSUM"))
        ps_scan = ab_stack.enter_context(tc.tile_pool(name="ps_scan", bufs=2, space="PSUM"))

        def ln_transpose_block(x_src, tok0, g_bc, b_bc, dstT):
            """DMA 4 x-tiles, LayerNorm, PE-transpose into dstT [P, D_T, BLK].
            Returns the list of x tiles (fp32) for residual use."""
            xts = []
            for t in range(ntt):
                x_t = xp.tile([P, DIM], f32, tag=f"x{t % 2}")
                nc.sync.dma_start(out=x_t, in_=x_src.ap()[tok0 + t * P:tok0 + (t + 1) * P, :])
                h_r = hrp.tile([P, DIM], bf16, tag=f"h{t % 2}")
                _layernorm_tile(nc, ln_w, x_t, g_bc, b_bc, eps_t, h_r)
                for fidx in range(D_T):
                    tps = ps_tp.tile([P, P], bf16, tag="tp_ps")
                    nc.tensor.transpose(tps, h_r[:, fidx * P:(fidx + 1) * P], ident)
                    nc.scalar.copy(out=dstT[:, fidx, t * P:(t + 1) * P], in_=tps)
                xts.append(x_t)
            return xts

        # ---------------- Phase A: prefix -> Mt ----------------
        for blk in range(n_pre_blk):
            hpT = h1Tp.tile([P, D_T, BLK], bf16, tag="h1T")
            ln_transpose_block(x_pre, blk * BLK, g1, b1, hpT)
            kc = actp.tile([P, ntt, DIM], bf16, tag="kc")
            vc = actp.tile([P, ntt, DIM], bf16, tag="vc", bufs=2)
            for (w_in, dst) in ((wk, kc), (wv, vc)):
                w_all = w_in.ap().rearrange("(kt p) d -> p kt d", p=P)
                for nh in range(2):
                    w_t = wsp.tile([P, D_T, 512], bf16, tag="w_t")
                    nc.sync.dma_start(out=w_t, in_=w_all[:, :, nh * 512:(nh + 1) * 512])
                    for m in range(ntt):
                        pst = ps_mm.tile([P, 512], f32, tag="pst")
                        for k in range(D_T):
                            nc.tensor.matmul(pst, hpT[:, k, m * P:(m + 1) * P], w_t[:, k, :],
                                             start=(k == 0), stop=(k == D_T - 1))
                        nc.scalar.copy(out=dst[:, m, nh * 512:(nh + 1) * 512], in_=pst)
            # Mt += kc^T vc per head, accumulated in PSUM across the block
            for h in range(HEADS):
                pb = (h % 2) * HD
                mt_ps = ps_scan.tile([HD, HD], f32, tag="kc" if h % 2 == 0 else "mt")
                for ch in range(ntt):
                    nc.tensor.matmul(mt_ps, kc[:, ch, h * HD:(h + 1) * HD],
                                     vc[:, ch, h * HD:(h + 1) * HD],
                                     start=(ch == 0), stop=(ch == ntt - 1))
                nc.vector.tensor_add(out=Mt_f[pb:pb + HD, h // 2, :],
                                     in0=Mt_f[pb:pb + HD, h // 2, :], in1=mt_ps)
        nc.scalar.copy(out=Mt_s, in_=Mt_f)

        # ---------------- Phase B: own tokens, attention ----------------
        scw = ab_stack.enter_context(tc.tile_pool(name="scw", bufs=3))
        x2fp = ab_stack.enter_context(tc.tile_pool(name="x2fp", bufs=2))
        yTp = ab_stack.enter_context(tc.tile_pool(name="yTp", bufs=2))
        for blk in range(n_blk):
            tok0 = blk * BLK
            h1T = h1Tp.tile([P, D_T, BLK], bf16, tag="h1T")
            ln_transpose_block(x_own, tok0, g1, b1, h1T)
            qT = actp.tile([P, D_T, BLK], bf16, tag="qT", bufs=2)
            kT = actp.tile([P, D_T, BLK], bf16, tag="kT", bufs=2)
            for (w_in, dst) in ((wq, qT), (wk, kT)):
                w_all = w_in.ap().rearrange("(kt p) d -> p kt d", p=P)
                for nh in range(2):
                    w_t = wsp.tile([P, D_T, 512], bf16, tag="w_t")
                    nc.sync.dma_start(out=w_t, in_=w_all[:, :, nh * 512:(nh + 1) * 512])
                    for ml in range(4):
                        m = nh * 4 + ml
                        pst = ps_mm.tile([P, BLK], f32, tag="pst")
                        for k in range(D_T):
                            nc.tensor.matmul(pst, w_t[:, k, ml * P:(ml + 1) * P], h1T[:, k, :],
                                             start=(k == 0), stop=(k == D_T - 1))
                        nc.scalar.copy(out=dst[:, m, :], in_=pst)
            v = actp.tile([P, ntt, DIM], bf16, tag="vc", bufs=2)
            wv_all = wv.ap().rearrange("(kt p) d -> p kt d", p=P)
            for nh in range(2):
                w_t = wsp.tile([P, D_T, 512], bf16, tag="w_t")
                nc.sync.dma_start(out=w_t, in_=wv_all[:, :, nh * 512:(nh + 1) * 512])
                for m in range(ntt):
                    pst = ps_mm.tile([P, 512], f32, tag="pst")
                    for k in range(D_T):
                        nc.tensor.matmul(pst, h1T[:, k, m * P:(m + 1) * P], w_t[:, k, :],
                                         start=(k == 0), stop=(k == D_T - 1))
                    nc.scalar.copy(out=v[:, m, nh * 512:(nh + 1) * 512], in_=pst)
            # scan: chunk-outer / head-inner so Mt chains are 16 apart
            y = actp.tile([P, ntt, DIM], bf16, tag="y")
            for ch in range(ntt):
                for h in range(HEADS):
                    pb = (h % 2) * HD
                    fi = h // 2
                    qcT = qT[pb:pb + HD, fi, ch * P:(ch + 1) * P]
                    kcT = kT[pb:pb + HD, fi, ch * P:(ch + 1) * P]
                    vc_s = v[:, ch, h * HD:(h + 1) * HD]
                    kc_ps = ps_scan.tile([P, HD], bf16, tag="kc")
                    nc.tensor.transpose(kc_ps, kcT, ident[pb:pb + HD, pb:pb + HD])
                    kc_s = scw.tile([P, HD], bf16, tag="kc_s")
                    nc.scalar.copy(out=kc_s, in_=kc_ps)
                    sc_ps = ps_tp.tile([P, P], f32, tag="tp_ps")
                    nc.tensor.matmul(sc_ps, kcT, qcT, start=True, stop=True)
                    sc_r = scw.tile([P, P], bf16, tag="sc_r")
                    nc.vector.tensor_mul(out=sc_r, in0=sc_ps, in1=mask_t)
                    y_ps = ps_mm.tile([P, HD], f32, tag="pst")
                    nc.tensor.matmul(y_ps, sc_r, vc_s, start=True, stop=False)
                    nc.tensor.matmul(y_ps, qcT, Mt_s[pb:pb + HD, fi, :], start=False, stop=True)
                    nc.scalar.copy(out=y[:, ch, h * HD:(h + 1) * HD], in_=y_ps)
                    mt_ps = ps_scan.tile([HD, HD], f32, tag="mt")
                    nc.tensor.matmul(mt_ps, kc_s, vc_s, start=True, stop=True)
                    nc.vector.tensor_add(out=Mt_f[pb:pb + HD, fi, :],
                                         in0=Mt_f[pb:pb + HD, fi, :], in1=mt_ps)
                    nc.scalar.copy(out=Mt_s[pb:pb + HD, fi, :], in_=Mt_f[pb:pb + HD, fi, :])
            # epilogue: yT, attn-out + residual, LN2, h2T into persistent hT
            for m in range(ntt):
                yT_m = yTp.tile([P, D_T, P], bf16, tag="yT_m")
                for fidx in range(D_T):
                    tps = ps_tp.tile([P, P], bf16, tag="tp_ps")
                    nc.tensor.transpose(tps, y[:, m, fidx * P:(fidx + 1) * P], ident)
                    nc.scalar.copy(out=yT_m[:, fidx, :], in_=tps)
                x_t = xp.tile([P, DIM], f32, tag=f"x{m % 2}")
                nc.sync.dma_start(out=x_t, in_=x_own.ap()[tok0 + m * P:tok0 + (m + 1) * P, :])
                ti = (tok0 // P) + m
                x2f = x2fp.tile([P, DIM], f32, tag="x2f")
                for nh in range(2):
                    pst = ps_mm.tile([P, 512], f32, tag="pst")
                    for k in range(D_T):
                        nc.tensor.matmul(pst, yT_m[:, k, :], wo_sb[:, k, nh * 512:(nh + 1) * 512],
                                         start=(k == 0), stop=(k == D_T - 1))
                    nc.vector.tensor_add(out=x2f[:, nh * 512:(nh + 1) * 512],
                                         in0=x_t[:, nh * 512:(nh + 1) * 512], in1=pst)
                x2b = x2fp.tile([P, DIM], bf16, tag="x2b")
                nc.scalar.copy(out=x2b, in_=x2f)
                nc.scalar.dma_start(out=x2_d[ti], in_=x2b)
                h2_r = hrp.tile([P, DIM], bf16, tag="h2r")
                _layernorm_tile(nc, ln_w, x2f, g2, b2, eps_t, h2_r)
                for fidx in range(D_T):
                    tps = ps_tp.tile([P, P], bf16, tag="tp_ps")
                    nc.tensor.transpose(tps, h2_r[:, fidx * P:(fidx + 1) * P], ident)
                    nc.scalar.copy(out=hT[:, fidx, ti * P:(ti + 1) * P], in_=tps)

        ab_stack.close()

        # final level's output in bf16 (fp8 PE transpose is illegal, so
        # phase D transposes this instead of hT)
        hfin_pool = top.enter_context(tc.tile_pool(name="hfin", bufs=1))
        h_fin = hfin_pool.tile([P, D_T, t_own], bf16)

        # ---------------- Phase C: CMS FFN, fully SBUF-resident ----------------
        with ExitStack() as ffn:
            big = ffn.enter_context(tc.tile_pool(name="ffn_big", bufs=1))
            upg = big.tile([P, HHT, t_own], f8)         # gelu acts, one hidden half
            out_acc = big.tile([P, D_T, t_own], bf16)   # half-0 partials + b2 (true scale)
            bp = ffn.enter_context(tc.tile_pool(name="ffn_b", bufs=2))
            w1s = ffn.enter_context(tc.tile_pool(name="w1s", bufs=2))
            w2s = ffn.enter_context(tc.tile_pool(name="w2s", bufs=2))
            hsc = ffn.enter_context(tc.tile_pool(name="hsc", bufs=3))
            ps_up = ffn.enter_context(tc.tile_pool(name="ps_up", bufs=4, space="PSUM"))
            ps_dn = ffn.enter_context(tc.tile_pool(name="ps_dn", bufs=4, space="PSUM"))
            for lvl in range(LEVELS):
                b1_t = bp.tile([P, H_T], f32, tag="b1")
                nc.sync.dma_start(out=b1_t, in_=cms_b1.ap()[lvl].rearrange("(m p) -> p m", p=P))
                b2_t = bp.tile([P, D_T], f32, tag="b2")
                nc.sync.dma_start(out=b2_t, in_=cms_b2.ap()[lvl].rearrange("(m p) -> p m", p=P))
                w1_all = cms_w1.ap()[lvl].rearrange("(kt p) d -> p kt d", p=P)
                w2_all = cms_w2.ap()[lvl].rearrange("(kt p) d -> p kt d", p=P)
                for half in range(2):
                    for mg in range(4):  # 512 hidden cols per w1 tile
                        w1_t = w1s.tile([P, D_T, 512], f8, tag="w1t")
                        c0 = half * (HID // 2) + mg * 512
                        nc.sync.dma_start(out=w1_t, in_=w1_all[:, :, c0:c0 + 512])
                        for ml in range(4):
                            mh = mg * 4 + ml           # hidden tile within half
                            m_gl = half * HHT + mh     # global hidden tile
                            psl = [ps_up.tile([P, TT], f32, tag="up",
                                              name=f"up_{lvl}_{half}_{mh}_{i}")
                                   for i in range(n_tt)]
                            for kk in range(D_T // 2):
                                for tt in range(n_tt):
                                    nc.tensor.matmul(
                                        psl[tt], w1_t[:, 2 * kk:2 * kk + 2, ml * P:(ml + 1) * P],
                                        hT[:, 2 * kk:2 * kk + 2, tt * TT:(tt + 1) * TT],
                                        start=(kk == 0), stop=(kk == D_T // 2 - 1),
                                        perf_mode=PM.DoubleRow)
                            for tt in range(n_tt):
                                nc.scalar.activation(
                                    out=upg[:, mh, tt * TT:(tt + 1) * TT], in_=psl[tt],
                                    func=AF.Gelu_apprx_tanh,
                                    bias=b1_t[:, m_gl:m_gl + 1], scale=INV_W8)
                    for mdg in range(4):  # 256 output cols per w2 tile
                        w2_t = w2s.tile([P, HHT, 256], f8, tag="w2t")
                        nc.sync.dma_start(
                            out=w2_t,
                            in_=w2_all[:, half * HHT:(half + 1) * HHT, mdg * 256:(mdg + 1) * 256])
                        for mdl in range(2):
                            md = mdg * 2 + mdl
                            psl = [ps_dn.tile([P, TT], f32, tag="dn",
                                              name=f"dn_{lvl}_{half}_{md}_{i}")
                                   for i in range(n_tt)]
                            for kk in range(HHT // 2):
                                for tt in range(n_tt):
                                    nc.tensor.matmul(
                                        psl[tt], w2_t[:, 2 * kk:2 * kk + 2, mdl * P:(mdl + 1) * P],
                                        upg[:, 2 * kk:2 * kk + 2, tt * TT:(tt + 1) * TT],
                                        start=(kk == 0), stop=(kk == HHT // 2 - 1),
                                        perf_mode=PM.DoubleRow)
                            for tt in range(n_tt):
                                if half == 0:
                                    nc.scalar.activation(
                                        out=out_acc[:, md, tt * TT:(tt + 1) * TT], in_=psl[tt],
                                        func=AF.Identity, bias=b2_t[:, md:md + 1], scale=INV_W8)
                                else:
                                    htmp = hsc.tile([P, TT], bf16, tag="htmp")
                                    nc.scalar.activation(out=htmp, in_=psl[tt], func=AF.Identity,
                                                         bias=0.0, scale=INV_W8)
                                    dst = h_fin if lvl == LEVELS - 1 else hT
                                    nc.vector.tensor_add(
                                        out=dst[:, md, tt * TT:(tt + 1) * TT],
                                        in0=htmp, in1=out_acc[:, md, tt * TT:(tt + 1) * TT])

        # ---------------- Phase D: out = x2 + h^T ----------------
        with ExitStack() as ph:
            sb = ph.enter_context(tc.tile_pool(name="D_sb", bufs=3))
            ps = ph.enter_context(tc.tile_pool(name="D_ps", bufs=3, space="PSUM"))
            for t in range(n_own_t):
                x2_t = sb.tile([P, DIM], bf16, tag="D_x2")
                nc.sync.dma_start(out=x2_t, in_=x2_d[t])
                o_t = sb.tile([P, DIM], f32, tag="D_o")
                for fidx in range(D_T):
                    tps = ps.tile([P, P], bf16, tag="D_tp")
                    nc.tensor.transpose(tps, h_fin[:, fidx, t * P:(t + 1) * P], ident)
                    nc.vector.tensor_add(out=o_t[:, fidx * P:(fidx + 1) * P],
                                         in0=x2_t[:, fidx * P:(fidx + 1) * P],
                                         in1=tps)
                nc.scalar.dma_start(out=out.ap()[t * P:(t + 1) * P, :], in_=o_t)

    _split_multi_waits(nc)
    return nc


_NC_CACHE = {}
LAST_RESULT = None


def _get_nc(key, **kw):
    if key not in _NC_CACHE:
        _NC_CACHE[key] = build_kernel(**kw)
    return _NC_CACHE[key]


def kernel(x, ln1_g, ln1_b, wq, wk, wv, wo, ln2_g, ln2_b,
           cms_w1, cms_b1, cms_w2, cms_b2, **extra):
    import ml_dtypes
    bf = ml_dtypes.bfloat16
    f8h = ml_dtypes.float8_e4m3
    x = np.asarray(x, np.float32)
    maskT = np.triu(np.ones((CHUNK, CHUNK), np.float32))  # maskT[e,c] = e<=c
    common = {
        "wq": np.asarray(wq, bf), "wk": np.asarray(wk, bf),
        "wv": np.asarray(wv, bf), "wo": np.asarray(wo, bf),
        "ln1_g": np.asarray(ln1_g, np.float32), "ln1_b": np.asarray(ln1_b, np.float32),
        "ln2_g": np.asarray(ln2_g, np.float32), "ln2_b": np.asarray(ln2_b, np.float32),
        "cms_w1": (np.asarray(cms_w1, np.float32) * W8_SCALE).astype(f8h),
        "cms_b1": np.asarray(cms_b1, np.float32),
        "cms_w2": (np.asarray(cms_w2, np.float32) * W8_SCALE).astype(f8h),
        "cms_b2": np.asarray(cms_b2, np.float32),
        "maskT": maskT,
    }
    zeros_pre = np.zeros((T_PRE, DIM), np.float32)
    in_maps = []
    for c in range(N_CORES):
        b, half = c // 2, c % 2
        own = x[b, half * T_OWN:(half + 1) * T_OWN]
        pre = x[b, 0:T_PRE] if half else zeros_pre
        in_maps.append({**common, "x_own": np.ascontiguousarray(own),
                        "x_pre": np.ascontiguousarray(pre)})
    nc = _get_nc("full")
    res = run_bass_kernel_spmd(nc, in_maps, core_ids=list(range(N_CORES)))
    global LAST_RESULT
    LAST_RESULT = res
    out = np.empty((B, S, DIM), np.float32)
    for c in range(N_CORES):
        b, half = c // 2, c % 2
        out[b, half * T_OWN:(half + 1) * T_OWN] = res.results[c]["out"]
    return out
